# revision 1
# baseline (speedup 1.0000x reference)
"""3-layer GCN forward (GCNConv x3 + log_softmax) on 8 Trainium2 cores.

Strategy (self-contained; shapes hardcoded for N=100000, Cin=Ch=128,
Cout=47, 8 cores):
  A_hat = D^-1/2 (A+I) D^-1/2 is fixed across layers, so per layer
      out = dinv_dst * segsum_dst( dinv_src * (H @ W) ) + b
  Host: permute nodes into 8 contiguous core blocks (degree-sorted within
  each block), build per-core padded gather grids: 98 groups of 128 output
  rows, each with d_g gather steps (shared loop structure across cores).
  Device (SPMD, one NEFF on cores 0-7):
    per layer: tiled GEMM + dinv_src row scale -> local Z block;
    AllGather Z across the 8 cores into a shared DRAM replica;
    aggregation: per group, d_g indirect-DMA row gathers (128 rows/instr)
    accumulated on the tensor engine via identity-matmul into PSUM;
    then dinv_dst scale + bias + relu (or log_softmax on the last layer).

z_full row space: node (core k, local r) lives at row k*12544 + r; rows
[12500, 12544) of each block are zero pads; ZROW (=12500) backs unused
grid slots.
"""
import numpy as np

NCORES = 8
N = 100000
NBLK = 12500
NPAD = 12544            # 98 * 128
NGRP = NPAD // 128      # 98
C = 128
COUT = 47
ZROW = NBLK             # a zero pad row (core 0 block)


def _preprocess(x, edge_index, W1, b1, W2, b2, W3, b3):
    x = np.asarray(x, np.float32)
    ei = np.asarray(edge_index)
    loop = np.arange(N, dtype=np.int64)
    src = np.concatenate([ei[0], loop]).astype(np.int64)
    dst = np.concatenate([ei[1], loop]).astype(np.int64)

    deg = np.bincount(dst, minlength=N).astype(np.float32)
    dinv = 1.0 / np.sqrt(np.maximum(deg, 1.0))

    # deal degree-ranked nodes round-robin across cores so all 8 cores'
    # group degree profiles align (minimizes cross-core max padding)
    rank = np.argsort(-deg, kind="stable")
    perm = np.empty(N, np.int64)
    for k in range(NCORES):
        perm[k * NBLK:(k + 1) * NBLK] = rank[k::NCORES]
    inv = np.empty(N, np.int64)
    inv[perm] = np.arange(N)

    srcp = inv[src]
    dstp = inv[dst]
    ksrc = srcp // NBLK
    srcg = ksrc * NPAD + (srcp - ksrc * NBLK)     # padded-global coords

    dinv_p = dinv[perm]

    ecore = dstp // NBLK
    rloc = dstp - ecore * NBLK
    order = np.lexsort((srcg, rloc, ecore))
    ecore, rloc, srcg_s = ecore[order], rloc[order], srcg[order]

    flat = ecore * NBLK + rloc                     # sorted
    cnt = np.bincount(flat, minlength=NCORES * NBLK)
    cnt_pad = np.zeros(NCORES * NPAD, np.int64)
    idx_all = (np.arange(NCORES * NBLK) // NBLK) * NPAD + \
        (np.arange(NCORES * NBLK) % NBLK)
    cnt_pad[idx_all] = cnt
    d_per = cnt_pad.reshape(NCORES, NGRP, 128).max(axis=2)
    d_g = np.maximum(d_per.max(axis=0), 1).astype(np.int64)
    col_off = np.concatenate([[0], np.cumsum(d_g)])
    n_steps = int(col_off[-1])

    tables = np.full((NCORES, 128, n_steps), ZROW, np.int32)
    starts = np.zeros(NCORES * NBLK + 1, np.int64)
    np.cumsum(cnt, out=starts[1:])
    pos = np.arange(len(order)) - starts[flat]
    grp = rloc // 128
    part = rloc % 128
    colidx = col_off[grp] + pos
    tables[ecore, part, colidx] = srcg_s.astype(np.int32)

    dinv_loc = np.zeros((NCORES, 128, NGRP), np.float32)
    dv = dinv_p.reshape(NCORES, NBLK)
    for k in range(NCORES):
        full = np.zeros(NPAD, np.float32)
        full[:NBLK] = dv[k]
        dinv_loc[k] = full.reshape(NGRP, 128).T

    xp = x[perm]
    xblk = np.zeros((NCORES, NPAD, C), np.float32)
    for k in range(NCORES):
        xblk[k, :NBLK] = xp[k * NBLK:(k + 1) * NBLK]

    Ws = [np.ascontiguousarray(W, np.float32) for W in (W1, W2, W3)]
    bb = [np.tile(np.asarray(b, np.float32)[None, :], (128, 1))
          for b in (b1, b2, b3)]

    in_maps = []
    for k in range(NCORES):
        in_maps.append({
            "xblk": np.ascontiguousarray(xblk[k]),
            "gidx": np.ascontiguousarray(tables[k]),
            "dinv": np.ascontiguousarray(dinv_loc[k]),
            "w1": Ws[0], "w2": Ws[1], "w3": Ws[2],
            "bb1": np.ascontiguousarray(bb[0]),
            "bb2": np.ascontiguousarray(bb[1]),
            "bb3": np.ascontiguousarray(bb[2]),
        })
    return in_maps, [int(v) for v in d_g], n_steps, perm


def _build(d_g, n_steps):
    from concourse import bacc, bass, mybir, tile
    from concourse.masks import make_identity
    f32 = mybir.dt.float32
    i32 = mybir.dt.int32
    couts = [C, C, COUT]

    nc = bacc.Bacc("TRN2", target_bir_lowering=False, debug=False,
                   num_devices=NCORES)
    xblk = nc.dram_tensor("xblk", [NPAD, C], f32, kind="ExternalInput")
    gidx = nc.dram_tensor("gidx", [128, n_steps], i32, kind="ExternalInput")
    dinv = nc.dram_tensor("dinv", [128, NGRP], f32, kind="ExternalInput")
    w_in = [nc.dram_tensor(f"w{l+1}", [C, couts[l]], f32,
                           kind="ExternalInput") for l in range(3)]
    bb_in = [nc.dram_tensor(f"bb{l+1}", [128, couts[l]], f32,
                            kind="ExternalInput") for l in range(3)]
    out_d = nc.dram_tensor("out", [NPAD, COUT], f32, kind="ExternalOutput")

    zsA = nc.dram_tensor("zsA", [NPAD, C], f32)          # layers 0,1
    zsB = nc.dram_tensor("zsB", [NPAD, COUT], f32)       # layer 2
    zf = [nc.dram_tensor(f"zf{l}", [NCORES * NPAD, couts[l]], f32,
                         addr_space="Shared") for l in range(3)]

    with tile.TileContext(nc) as tc:
        with tc.tile_pool(name="const", bufs=1) as cpool, \
             tc.tile_pool(name="hbuf", bufs=1) as hpool, \
             tc.tile_pool(name="gath", bufs=10) as gpool, \
             tc.tile_pool(name="work", bufs=4) as wpool, \
             tc.tile_pool(name="ps_t", bufs=2, space="PSUM") as ps_t, \
             tc.tile_pool(name="ps_z", bufs=2, space="PSUM") as ps_z, \
             tc.tile_pool(name="ps_g", bufs=2, space="PSUM") as ps_g:

            ident = cpool.tile([128, 128], f32)
            make_identity(nc, ident[:])
            idx_sb = cpool.tile([128, n_steps], i32)
            nc.sync.dma_start(out=idx_sb[:], in_=gidx[:])
            dinv_sb = cpool.tile([128, NGRP], f32)
            nc.sync.dma_start(out=dinv_sb[:], in_=dinv[:])
            w_sb, bb_sb = [], []
            for l in range(3):
                w = cpool.tile([128, couts[l]], f32, name=f"w_sb{l}")
                nc.sync.dma_start(out=w[:], in_=w_in[l][:])
                w_sb.append(w)
                b = cpool.tile([128, couts[l]], f32, name=f"bb_sb{l}")
                nc.sync.dma_start(out=b[:], in_=bb_in[l][:])
                bb_sb.append(b)

            H = hpool.tile([128, NGRP * C], f32)

            for lay in range(3):
                co = couts[lay]
                zs = zsA if lay < 2 else zsB
                for g in range(NGRP):
                    if lay == 0:
                        hin = wpool.tile([128, C], f32, name="hin")
                        nc.sync.dma_start(
                            out=hin[:], in_=xblk[g * 128:(g + 1) * 128, :])
                        hsrc = hin[:]
                    else:
                        hsrc = H[:, g * C:(g + 1) * C]
                    pst = ps_t.tile([128, 128], f32, name="pst")
                    nc.tensor.transpose(out=pst[:], in_=hsrc, identity=ident[:])
                    ht = wpool.tile([128, 128], f32, name="ht")
                    nc.vector.tensor_copy(out=ht[:], in_=pst[:])
                    psz = ps_z.tile([128, co], f32, name="psz")
                    nc.tensor.matmul(out=psz[:], lhsT=ht[:], rhs=w_sb[lay][:],
                                     start=True, stop=True)
                    zt = wpool.tile([128, C], f32, name="zt")
                    nc.vector.tensor_scalar_mul(out=zt[:, :co], in0=psz[:],
                                                scalar1=dinv_sb[:, g:g + 1])
                    nc.sync.dma_start(out=zs[g * 128:(g + 1) * 128, :],
                                      in_=zt[:, :co])

                nc.gpsimd.collective_compute(
                    "AllGather", mybir.AluOpType.bypass,
                    replica_groups=[list(range(NCORES))],
                    ins=[zs[:, :]], outs=[zf[lay][:, :]])

                s = 0
                for g in range(NGRP):
                    d = d_g[g]
                    nq = min(4, d)
                    psg = ps_g.tile([128, 4 * C], f32, name="psg")
                    nch = (d + 3) // 4
                    jj = 0
                    for ch in range(nch):
                        w = min(4, d - jj)
                        gs4 = gpool.tile([128, 4 * C], f32, name="gs")
                        for q in range(w):
                            nc.gpsimd.indirect_dma_start(
                                out=gs4[:, q * C:q * C + co], out_offset=None,
                                in_=zf[lay][:, :],
                                in_offset=bass.IndirectOffsetOnAxis(
                                    ap=idx_sb[:, s:s + 1], axis=0))
                            s += 1
                        nc.tensor.matmul(out=psg[:, :w * C], lhsT=ident[:],
                                         rhs=gs4[:, :w * C],
                                         start=(ch == 0), stop=(ch == nch - 1))
                        jj += w
                    tmp = wpool.tile([128, C], f32, name="tmp")
                    nc.vector.tensor_copy(out=tmp[:, :co], in_=psg[:, :co])
                    for q in range(1, nq):
                        nc.vector.tensor_add(out=tmp[:, :co], in0=tmp[:, :co],
                                             in1=psg[:, q * C:q * C + co])
                    nc.vector.tensor_scalar_mul(out=tmp[:, :co], in0=tmp[:, :co],
                                                scalar1=dinv_sb[:, g:g + 1])
                    nc.vector.tensor_add(out=tmp[:, :co], in0=tmp[:, :co],
                                         in1=bb_sb[lay][:])
                    if lay < 2:
                        nc.vector.tensor_scalar_max(
                            out=H[:, g * C:(g + 1) * C], in0=tmp[:, :co],
                            scalar1=0.0)
                    else:
                        mx = wpool.tile([128, 1], f32, name="mx")
                        nc.vector.tensor_reduce(
                            out=mx[:], in_=tmp[:, :co],
                            axis=mybir.AxisListType.X, op=mybir.AluOpType.max)
                        nmx = wpool.tile([128, 1], f32, name="nmx")
                        nc.vector.tensor_scalar_mul(out=nmx[:], in0=mx[:],
                                                    scalar1=-1.0)
                        ex = wpool.tile([128, C], f32, name="ex")
                        ssum = wpool.tile([128, 1], f32, name="ssum")
                        nc.scalar.activation(
                            out=ex[:, :co], in_=tmp[:, :co],
                            func=mybir.ActivationFunctionType.Exp,
                            bias=nmx[:], scale=1.0, accum_out=ssum[:])
                        lse = wpool.tile([128, 1], f32, name="lse")
                        nc.scalar.activation(
                            out=lse[:], in_=ssum[:],
                            func=mybir.ActivationFunctionType.Ln)
                        tot = wpool.tile([128, 1], f32, name="tot")
                        nc.vector.tensor_add(out=tot[:], in0=lse[:], in1=mx[:])
                        ot = wpool.tile([128, COUT], f32, name="ot")
                        nc.vector.tensor_scalar_sub(out=ot[:], in0=tmp[:, :co],
                                                    scalar1=tot[:])
                        nc.sync.dma_start(
                            out=out_d[g * 128:(g + 1) * 128, :], in_=ot[:])

    nc.compile()
    return nc


def kernel(x, edge_index, W1, b1, W2, b2, W3, b3):
    from concourse.bass_utils import run_bass_kernel_spmd

    in_maps, d_g, n_steps, perm = _preprocess(
        x, edge_index, W1, b1, W2, b2, W3, b3)
    nc = _build(d_g, n_steps)
    res = run_bass_kernel_spmd(nc, in_maps, core_ids=list(range(NCORES)))
    blocks = [res.results[k]["out"][:NBLK] for k in range(NCORES)]
    outp = np.concatenate(blocks, axis=0)
    out = np.empty((N, COUT), np.float32)
    out[perm] = outp
    return out

